# revision 1
# baseline (speedup 1.0000x reference)
"""Fused causal attention block (qkv proj + RoPE + attention + out proj) on 8 TRN2 cores.

Sharding: data-parallel over batch (2) x tensor-parallel over heads (16 -> 4 per core).
Each core computes y_partial[b] = attn_heads_group(x[b]) @ out_w[group_rows]; the host
sums the 4 partials per batch (the out-projection "all-reduce") and stacks batches.

Device kernel (per core, Tile framework):
  phase A: load xT/weights, compute qT,kT (transposed [d_head, s] layout) and v (natural),
           RoPE applied to q,k in transposed layout via paired partition-slice DVE ops.
  phase B: per head, per 512-wide q chunk: scores^T = k_tile^T @ q (PSUM, fp32r),
           causal mask added via identity-matmul bias injection, exp on ACT (scale=1/8,
           no max subtraction -- logits are O(5) by construction), PV accumulation with a
           ones-column appended to v so the softmax denominator falls out of the same
           matmul, normalization via K=1 broadcast matmul + DVE multiply.
  phase C: y = attnT^T @ wo per 128-row s-tile, DMA out.
"""

import numpy as np

S = 2048
D = 1024
H = 16
DH = 64
P = 128
HPC = 4          # heads per core
QC = 512         # q-chunk width
NQC = S // QC
NKT = S // P     # k tiles
DIN_T = D // P   # contraction tiles for projections
NST = S // P     # s tiles
MBIG = -240000.0  # pre-scale mask bias; * 0.125 = -30000 -> exp == 0.0


def _build_nc(is_causal: bool, use_kbias: bool):
    import concourse.bass as bass
    import concourse.mybir as mybir
    import concourse.tile as tile

    f32 = mybir.dt.float32
    f32r = mybir.dt.float32r
    EXP = mybir.ActivationFunctionType.Exp

    nc = bass.Bass()
    wfix_sem = nc.alloc_semaphore("wfix")
    xT = nc.dram_tensor("xT", [D, S], f32r, kind="ExternalInput")
    wq = nc.dram_tensor("wq", [D, 256], f32r, kind="ExternalInput")
    wk = nc.dram_tensor("wk", [D, 256], f32r, kind="ExternalInput")
    wv = nc.dram_tensor("wv", [D, 256], f32r, kind="ExternalInput")
    wo = nc.dram_tensor("wo", [HPC, 64, D], f32r, kind="ExternalInput")
    ctab = nc.dram_tensor("ctab", [P, S], mybir.dt.float16, kind="ExternalInput")
    ttab = nc.dram_tensor("ttab", [P, S], mybir.dt.float16, kind="ExternalInput")
    mask = nc.dram_tensor("mask", [P, 896], f32r, kind="ExternalInput")
    ident = nc.dram_tensor("ident", [P, P], f32r, kind="ExternalInput")
    kbias = nc.dram_tensor("kbias", [1, S], f32r, kind="ExternalInput")
    ones_in = nc.dram_tensor("ones_in", [65, 64], f32r, kind="ExternalInput")
    vones = nc.dram_tensor("vones", [P, NKT], f32r, kind="ExternalInput")
    y = nc.dram_tensor("y", [S, D], f32, kind="ExternalOutput")

    with tile.TileContext(nc) as tc, nc.allow_low_precision(
        reason="fp32r is bit-identical to fp32 here; matmul inputs must be typed fp32r"
    ):
        with (
            tc.tile_pool(name="pers", bufs=1) as pers,
            tc.tile_pool(name="ropet", bufs=2) as ropet,
        ):
            qT_sb = pers.tile([P, 2, S], f32r, tag="qT")
            kT_sb = pers.tile([P, 2, S], f32r, tag="kT")
            v_sb = pers.tile([P, HPC, NKT, 65], f32r, tag="v")
            attnT_sb = pers.tile([64, HPC, S], f32r, tag="attnT")
            wo_sb = pers.tile([64, HPC, D], f32r, tag="wo")
            mask_sb = pers.tile([P, 896], f32r, tag="mask")
            ident_sb = pers.tile([P, P], f32r, tag="ident")
            ones_sb = pers.tile([65, 64], f32r, tag="ones")
            if use_kbias:
                kbias_sb = pers.tile([1, S], f32r, tag="kbias")
                ones_q = pers.tile([1, QC], f32r, tag="onesq")
                nc.sync.dma_start(out=kbias_sb, in_=kbias[:, :])
                nc.vector.memset(ones_q, 1.0)



            with (
                tc.tile_pool(name="pha", bufs=1) as pha,
                tc.tile_pool(name="xpool", bufs=3) as xpool,
                tc.tile_pool(name="projps", bufs=4, space="PSUM") as projps,
                tc.tile_pool(name="vps", bufs=2, space="PSUM") as vps,
            ):
                wq_sb = pha.tile([P, DIN_T, 256], f32r, tag="wq")
                wk_sb = pha.tile([P, DIN_T, 256], f32r, tag="wk")
                wv_sb = pha.tile([P, DIN_T, 256], f32r, tag="wv")
                c_sb = pha.tile([P, S], mybir.dt.float16, tag="ctab")
                t_sb = pha.tile([P, S], mybir.dt.float16, tag="ttab")

                for sc in range(NQC):
                    xTc = xpool.tile([P, DIN_T, QC], f32r, tag="xTc")
                    for kc in range(DIN_T):
                        nc.sync.dma_start(
                            out=xTc[:, kc, :],
                            in_=xT[kc * P:(kc + 1) * P, sc * QC:(sc + 1) * QC],
                        )
                        if sc == 0:
                            nc.sync.dma_start(out=wq_sb[:, kc, :], in_=wq[kc * P:(kc + 1) * P, :])
                            nc.sync.dma_start(out=wk_sb[:, kc, :], in_=wk[kc * P:(kc + 1) * P, :])
                            nc.sync.dma_start(out=wv_sb[:, kc, :], in_=wv[kc * P:(kc + 1) * P, :])
                    if sc == 0:
                        nc.sync.dma_start(out=c_sb, in_=ctab[:, :])
                        nc.sync.dma_start(out=t_sb, in_=ttab[:, :])
                    # q/k projections + RoPE for this s-chunk
                    cs = c_sb[:, sc * QC:(sc + 1) * QC]
                    ts = t_sb[:, sc * QC:(sc + 1) * QC]
                    for dst, w_sb in ((qT_sb, wq_sb), (kT_sb, wk_sb)):
                        for X in range(2):
                            pq = projps.tile([P, QC], f32, tag="pq")
                            for kc in range(DIN_T):
                                nc.tensor.matmul(
                                    pq,
                                    w_sb[:, kc, X * P:(X + 1) * P],
                                    xTc[:, kc, :],
                                    start=(kc == 0),
                                    stop=(kc == DIN_T - 1),
                                )
                            tmp = ropet.tile([P, QC], f32, tag="tmp")
                            z = ropet.tile([P, QC], mybir.dt.float16, tag="z")
                            zs = ropet.tile([P, QC], mybir.dt.float16, tag="zs")
                            nc.vector.tensor_mul(tmp, pq, cs)
                            nc.vector.tensor_mul(z, pq, ts)
                            for blk in (0, 64):
                                nc.sync.dma_start(out=zs[blk:blk + 32, :], in_=z[blk + 32:blk + 64, :])
                                nc.sync.dma_start(out=zs[blk + 32:blk + 64, :], in_=z[blk:blk + 32, :])
                            dv = dst[:, X, sc * QC:(sc + 1) * QC]
                            nc.vector.tensor_add(dv, tmp, zs)
                    # v projection for the 4 s-tiles of this chunk
                    for j in range(4):
                        st = sc * 4 + j
                        pv = vps.tile([P, 256], f32, tag="pv")
                        for kc in range(DIN_T):
                            nc.tensor.matmul(
                                pv,
                                xTc[:, kc, j * P:(j + 1) * P],
                                wv_sb[:, kc, :],
                                start=(kc == 0),
                                stop=(kc == DIN_T - 1),
                            )
                        nc.scalar.copy(
                            out=v_sb[:, :, st, 0:64],
                            in_=pv.rearrange("p (h c) -> p h c", h=HPC),
                        )

            for h in range(HPC):
                nc.sync.dma_start(out=wo_sb[:, h, :], in_=wo[h, :, :])
            nc.sync.dma_start(out=mask_sb, in_=mask[:, :])
            nc.sync.dma_start(out=ident_sb, in_=ident[:, :])
            nc.sync.dma_start(out=ones_sb, in_=ones_in[:, :])
            for h in range(HPC):
                nc.sync.dma_start(out=v_sb[:, h, :, 64:65], in_=vones.rearrange("p (k o) -> p k o", o=1))
            # ---- attention + fused out-projection (qc-outer) ----
            with (
                tc.tile_pool(name="epool", bufs=2) as epool,
                tc.tile_pool(name="rpool", bufs=1) as rpool,
                tc.tile_pool(name="ypool", bufs=3) as ypool,
                tc.tile_pool(name="scps", bufs=2, space="PSUM") as scps,
                tc.tile_pool(name="pvps", bufs=2, space="PSUM") as pvps,
                tc.tile_pool(name="bcps", bufs=1, space="PSUM") as bcps,
                tc.tile_pool(name="yps", bufs=1, space="PSUM") as yps,
            ):
                for qc in range(NQC):
                    q0 = qc * QC
                    nkt = q0 // P + 4 if is_causal else NKT
                    npair = (nkt + 1) // 2
                    for h in range(HPC):
                        X, o = h // 2, 64 * (h % 2)
                        qh = qT_sb[o:o + 64, X, :]
                        kh = kT_sb[o:o + 64, X, :]
                        O = pvps.tile([65, QC], f32, tag="O")
                        for pr in range(npair):
                            sc2 = scps.tile([P, 2 * QC], f32, tag="sc2")
                            e2 = epool.tile([P, 2 * QC], f32r, tag="e2")
                            for half in range(2):
                                kt = 2 * pr + half
                                if kt >= nkt:
                                    continue
                                scv = sc2[:, half * QC:(half + 1) * QC]
                                diag = is_causal and kt * P >= q0
                                nc.tensor.matmul(
                                    scv,
                                    kh[:, kt * P:(kt + 1) * P],
                                    qh[:, q0:q0 + QC],
                                    start=True,
                                    stop=not (diag or use_kbias),
                                )
                                if use_kbias:
                                    nc.tensor.matmul(
                                        scv,
                                        kbias_sb[:, kt * P:(kt + 1) * P],
                                        ones_q,
                                        start=False,
                                        stop=not diag,
                                    )
                                if diag:
                                    d = kt * P - q0
                                    nc.tensor.matmul(
                                        scv,
                                        ident_sb,
                                        mask_sb[:, 384 - d:896 - d],
                                        start=False,
                                        stop=True,
                                    )
                            nc.scalar.activation(out=e2, in_=sc2, func=EXP, scale=0.125)
                            for half in range(2):
                                kt = 2 * pr + half
                                if kt >= nkt:
                                    continue
                                nc.tensor.matmul(
                                    O,
                                    v_sb[:, h, kt, :],
                                    e2[:, half * QC:(half + 1) * QC],
                                    start=(kt == 0),
                                    stop=(kt == nkt - 1),
                                )
                        at = attnT_sb[0:64, h, q0:q0 + QC]
                        nc.vector.tensor_copy(at, O[0:64, :])
                        r = rpool.tile([65, QC], f32r, tag="r")
                        nc.vector.reciprocal(r[64:65, :], O[64:65, :])
                        bc = bcps.tile([64, QC], f32, tag="bc")
                        nc.tensor.matmul(
                            bc, ones_sb[64:65, :], r[64:65, :],
                            start=True, stop=True,
                        )
                        nc.vector.tensor_mul(at, at, bc)
                    # out projection for this q-chunk's four s-tiles
                    for j in range(4):
                        st = qc * 4 + j
                        for nb in range(2):
                            yp = yps.tile([P, QC], f32, tag="yp")
                            for h in range(HPC):
                                nc.tensor.matmul(
                                    yp,
                                    attnT_sb[:, h, st * P:(st + 1) * P],
                                    wo_sb[:, h, nb * QC:(nb + 1) * QC],
                                    start=(h == 0),
                                    stop=(h == HPC - 1),
                                )
                            yt = ypool.tile([P, QC], f32, tag="yt")
                            if nb == 0:
                                nc.vector.tensor_copy(yt, yp)
                            else:
                                nc.scalar.copy(out=yt, in_=yp)
                            nc.sync.dma_start(
                                out=y[st * P:(st + 1) * P, nb * QC:(nb + 1) * QC], in_=yt
                            )

    _split_matmul_waits(nc, wfix_sem)
    return nc


def _split_matmul_waits(nc, wfix_sem):
    """Walrus's engine-instruction sync-wait slots are scarce (fp32r matmul
    takes exactly one; DVE/ACT structs also cap out). Leave one wait on the
    instruction and move the rest onto NoOps inserted just before it, each
    carrying a single wait."""
    import concourse.mybir as mybir
    import bass_rust

    n_fix = 0
    for blk in nc.m.functions[0].blocks:
        il = blk.instructions
        out = []
        changed = False
        for inst in il:
            si = inst.sync_info
            if si is not None and len(si.on_wait) > 1:
                merged = {}
                for w in si.on_wait:
                    k = (w.sync_type, w.id, w.wait_mode)
                    if (
                        k in merged
                        and w.wait_mode == "sem-ge-imm"
                        and w.wait_reg is None
                    ):
                        if w.wait_value > merged[k].wait_value:
                            merged[k] = w
                    elif k in merged:
                        merged[(k, len(merged))] = w
                    else:
                        merged[k] = w
                waits = list(merged.values())
                if len(waits) == 1:
                    si.on_wait = waits
                    out.append(inst)
                    continue
                for j, w in enumerate(waits[:-1]):
                    nop = mybir.InstNoOp(name=f"{inst.name}-wfix{j}")
                    nop.engine = inst.engine
                    upd = bass_rust.SyncUpdate(
                        sync_type="semaphore", id=wfix_sem.num,
                        ant_name=wfix_sem.name, update_mode="sem-inc",
                        update_value=1, update_reg=None,
                    )
                    nop.sync_info = bass_rust.SyncInfo(on_wait=[w], on_update=[upd])
                    out.append(nop)
                    n_fix += 1
                si.on_wait = [waits[-1]]
                changed = True
            out.append(inst)
        if changed:
            blk.instructions = out


def _host_tables():
    j = np.arange(32)
    inv_freq = (10000.0 ** (-j / 32.0)).astype(np.float64)
    ang = np.arange(S, dtype=np.float64)[:, None] * inv_freq[None, :]  # [S, 32]
    cosv = np.cos(ang).astype(np.float32).T   # [32, S]
    sinv = np.sin(ang).astype(np.float32).T
    C = np.empty((P, S), dtype=np.float32)
    T = np.empty((P, S), dtype=np.float32)
    for blk in (0, 64):
        C[blk:blk + 32] = cosv
        C[blk + 32:blk + 64] = cosv
        T[blk:blk + 32] = sinv          # lo rows carry +sin (headed to hi output)
        T[blk + 32:blk + 64] = -sinv    # hi rows carry -sin (headed to lo output)
    i = np.arange(P)[:, None]
    u = np.arange(896)[None, :]
    M = np.where(u >= i + 384, 0.0, MBIG).astype(np.float32)
    return C.astype(np.float16), T.astype(np.float16), M


def _in_maps(x, qkv_w, out_w, attn_mask, is_causal):
    C, T, M = _host_tables()
    ident = np.eye(P, dtype=np.float32)
    wq_full = qkv_w[:, 0:D]
    wk_full = qkv_w[:, D:2 * D]
    wv_full = qkv_w[:, 2 * D:3 * D]
    use_kbias = (not is_causal) and not bool(np.all(attn_mask))
    maps = []
    for core in range(8):
        b, hg = core // 4, core % 4
        cols = slice(hg * 256, (hg + 1) * 256)
        if use_kbias:
            kb = np.where(attn_mask[b], 0.0, MBIG).astype(np.float32)[None, :]
        else:
            kb = np.zeros((1, S), dtype=np.float32)
        maps.append(
            dict(
                xT=np.ascontiguousarray(x[b].T),
                wq=np.ascontiguousarray(wq_full[:, cols]),
                wk=np.ascontiguousarray(wk_full[:, cols]),
                wv=np.ascontiguousarray(wv_full[:, cols]),
                wo=np.ascontiguousarray(
                    out_w[hg * 256:(hg + 1) * 256, :].reshape(HPC, 64, D)
                ),
                ones_in=np.ones((65, 64), dtype=np.float32),
                vones=np.ones((P, NKT), dtype=np.float32),
                ctab=C,
                ttab=T,
                mask=M,
                ident=ident,
                kbias=kb,
            )
        )
    return maps, use_kbias


def kernel(x, qkv_w, out_w, attn_mask, is_causal):
    from concourse.bass_utils import run_bass_kernel_spmd

    x = np.asarray(x, dtype=np.float32)
    qkv_w = np.asarray(qkv_w, dtype=np.float32)
    out_w = np.asarray(out_w, dtype=np.float32)
    attn_mask = np.asarray(attn_mask).astype(bool)
    causal = bool(np.asarray(is_causal))

    maps, use_kbias = _in_maps(x, qkv_w, out_w, attn_mask, causal)
    nc = _build_nc(causal, use_kbias)
    res = run_bass_kernel_spmd(nc, maps, list(range(8)))
    out = np.zeros((2, S, D), dtype=np.float32)
    for core in range(8):
        out[core // 4] += res.results[core]["y"]
    return out



# revision 27
# speedup vs baseline: 1.3768x; 1.3768x over previous
"""Fused causal attention block (qkv proj + RoPE + attention + out proj) on 8 TRN2 cores.

Sharding: data-parallel over batch (2) x tensor-parallel over heads (16 -> 4 per core).
Each core computes y_partial[b] = attn_heads_group(x[b]) @ out_w[group_rows]; the host
sums the 4 partials per batch (the out-projection "all-reduce") and stacks batches.

v2 layout (all matmul operands fp16, fp32 PSUM accumulation):
  - chunk-interleaved schedule: proj(chunk i) || attn(chunk i-1); causal q-chunk i only
    needs k-chunks <= i, so attention starts while later projections still run.
  - diagonal k-tiles computed at trimmed width (only q >= k-tile start) using PSUM
    pending-zero semantics; causal mask added as a 128-wide static lower-triangle
    table via identity matmul (N=128 instead of N=512 per diag tile).
  - out-projection packs head pairs: attnT [128=2x64 vdims, 2, S] against
    wo [128, 2, D] -> K=128 contraction, half the accumulation passes. Odd head's
    normalized attn rows are moved to partitions 64-127 by a small SBUF->SBUF DMA.
  - softmax denominator from a ones-column appended to v (row 64 of the PV psum);
    reciprocal on DVE, broadcast across 64 partitions by a K=1 matmul, applied in
    the same DVE multiply that writes attnT.
"""

import numpy as np

S = 2048
D = 1024
H = 16
DH = 64
P = 128
HPC = 4          # heads per core
QC = 512         # q-chunk width
NQC = S // QC
NKT = S // P     # k tiles
DIN_T = D // P   # contraction tiles for projections
MBIG = -60000.0  # pre-scale mask bias (fp16-safe); * 0.125 = -7500 -> exp == 0.0


DEBUG_NAMES = ["dbg_qT", "dbg_kT", "dbg_v", "dbg_at"]


def _build_nc(is_causal: bool, use_kbias: bool, debug: bool = False):
    import concourse.bass as bass
    import concourse.mybir as mybir
    import concourse.tile as tile

    f16 = mybir.dt.float16
    f32 = mybir.dt.float32
    f32r = mybir.dt.float32r
    EXP = mybir.ActivationFunctionType.Exp

    nc = bass.Bass()
    wfix_sem = nc.alloc_semaphore("wfix")
    xT = nc.dram_tensor("xT", [D, S], f16, kind="ExternalInput")
    wq = nc.dram_tensor("wq", [D, 256], f16, kind="ExternalInput")
    wk = nc.dram_tensor("wk", [D, 256], f16, kind="ExternalInput")
    wv = nc.dram_tensor("wv", [D, 256], f16, kind="ExternalInput")
    wo = nc.dram_tensor("wo", [2, P, D], f16, kind="ExternalInput")
    ctab = nc.dram_tensor("ctab", [P, S], f16, kind="ExternalInput")
    ttab = nc.dram_tensor("ttab", [P, S], f16, kind="ExternalInput")
    mstd = nc.dram_tensor("mstd", [P, P], f16, kind="ExternalInput")
    ident = nc.dram_tensor("ident", [P, P], f16, kind="ExternalInput")
    kbias = nc.dram_tensor("kbias", [1, S], f16, kind="ExternalInput")
    vones = nc.dram_tensor("vones", [P, NKT], f16, kind="ExternalInput")
    onesb = nc.dram_tensor("onesb", [1, 64], f32r, kind="ExternalInput")
    y = nc.dram_tensor("y", [S, D], f16, kind="ExternalOutput")
    if debug:
        dbg_qT = nc.dram_tensor("dbg_qT", [P, 2, S], f16, kind="ExternalOutput")
        dbg_kT = nc.dram_tensor("dbg_kT", [P, 2, S], f16, kind="ExternalOutput")
        dbg_v = nc.dram_tensor("dbg_v", [P, HPC, NKT, 65], f16, kind="ExternalOutput")
        dbg_at = nc.dram_tensor("dbg_at", [P, 2, S], f16, kind="ExternalOutput")

    with tile.TileContext(nc) as tc, nc.allow_low_precision(
        reason="fp16 operands with fp32 PSUM accumulation; rel-err budget 2e-2"
    ):
        with (
            tc.tile_pool(name="pers", bufs=1) as pers,
            tc.tile_pool(name="xpool", bufs=1) as xpool,
            tc.tile_pool(name="ropet", bufs=4) as ropet,
            tc.tile_pool(name="epool", bufs=6) as epool,
            tc.tile_pool(name="stage", bufs=4) as stage,
            tc.tile_pool(name="ytile", bufs=6) as ytile,
            tc.tile_pool(name="scps", bufs=2, space="PSUM") as scps,
            tc.tile_pool(name="pvps", bufs=2, space="PSUM") as pvps,
            tc.tile_pool(name="mmps", bufs=2, space="PSUM") as mmps,
        ):
            qT_sb = pers.tile([P, 2, S], f16, tag="qT")
            kT_sb = pers.tile([P, 2, S], f16, tag="kT")
            v_sb = pers.tile([P, HPC, NKT, 66], f16, tag="v")
            attnT_sb = pers.tile([P, 2, S], f16, tag="attnT")
            wo_sb = pers.tile([P, 2, D], f16, tag="wo")
            mstd_sb = pers.tile([P, P], f16, tag="mstd")
            ident_sb = pers.tile([P, P], f16, tag="ident")
            ones_bc = pers.tile([1, 64], f32r, tag="onesbc")
            wq_sb = pers.tile([P, DIN_T, 256], f16, tag="wq")
            wk_sb = pers.tile([P, DIN_T, 256], f16, tag="wk")
            wv_sb = pers.tile([P, DIN_T, 256], f16, tag="wv")
            c_sb = pers.tile([P, S], f16, tag="ctab")
            t_sb = pers.tile([P, S], f16, tag="ttab")
            x_ch = [
                xpool.tile([P, DIN_T, QC], f16, name=f"x{sc}", tag=f"x{sc}")
                for sc in range(NQC)
            ]
            if use_kbias:
                kbias_sb = pers.tile([1, S], f16, tag="kbias")
                ones_q = pers.tile([1, QC], f16, tag="onesq")
                nc.sync.dma_start(out=kbias_sb, in_=kbias[:, :])
                nc.vector.memset(ones_q, 1.0)
            nc.sync.dma_start(out=ones_bc, in_=onesb[:, :])

            # ---- prologue DMAs (one per tensor: HWDGE issue costs 625ns/DMA) ----
            # q-projection critical path first: x0, wq, then wk, rope tables,
            # wv, the chunk-1 prefetch, and the rest.
            def x_load(sc):
                nc.sync.dma_start(
                    out=x_ch[sc],
                    in_=xT[:, sc * QC:(sc + 1) * QC].rearrange(
                        "(t p) f -> p t f", p=P
                    ),
                )

            for q4 in range(4):
                nc.sync.dma_start(
                    out=x_ch[0][:, 2 * q4:2 * q4 + 2, :],
                    in_=xT[2 * q4 * P:(2 * q4 + 2) * P, 0:QC].rearrange(
                        "(t p) f -> p t f", p=P
                    ),
                )
                if q4 % 2 == 1:
                    nc.sync.dma_start(
                        out=wq_sb[:, 4 * (q4 // 2):4 * (q4 // 2) + 4, :],
                        in_=wq[4 * (q4 // 2) * P:(4 * (q4 // 2) + 4) * P, :].rearrange(
                            "(t p) n -> p t n", p=P
                        ),
                    )
            nc.sync.dma_start(out=wk_sb, in_=wk.rearrange("(t p) n -> p t n", p=P))
            nc.sync.dma_start(out=c_sb, in_=ctab[:, :])
            nc.sync.dma_start(out=t_sb, in_=ttab[:, :])
            nc.sync.dma_start(out=wv_sb, in_=wv.rearrange("(t p) n -> p t n", p=P))
            x_load(1)
            nc.sync.dma_start(out=mstd_sb, in_=mstd[:, :])
            nc.sync.dma_start(out=ident_sb, in_=ident[:, :])
            for g in range(2):
                nc.sync.dma_start(out=wo_sb[:, g, :], in_=wo[g, :, :])
            for h in range(HPC):
                nc.sync.dma_start(
                    out=v_sb[:, h, :, 64:65],
                    in_=vones.rearrange("p (k o) -> p k o", o=1),
                )

            def proj_units(sc):
                xTc = x_ch[sc]
                cs = c_sb[:, sc * QC:(sc + 1) * QC]
                ts = t_sb[:, sc * QC:(sc + 1) * QC]

                def prefetch():
                    if sc + 2 < NQC:
                        x_load(sc + 2)

                def qk_unit(dst, w_sb, X):
                    def run():
                        pq = mmps.tile([P, QC], f32, tag="mm")
                        for kc in range(DIN_T):
                            nc.tensor.matmul(
                                pq,
                                w_sb[:, kc, X * P:(X + 1) * P],
                                xTc[:, kc, :],
                                start=(kc == 0),
                                stop=(kc == DIN_T - 1),
                            )
                        tmp = ropet.tile([P, QC], f16, tag="tmp")
                        z = ropet.tile([P, QC], f16, tag="z")
                        zs = ropet.tile([P, QC], f16, tag="zs")
                        nc.vector.tensor_mul(tmp, pq, cs)
                        nc.vector.tensor_mul(z, pq, ts)
                        for blk in (0, 64):
                            nc.sync.dma_start(
                                out=zs[blk:blk + 32, :], in_=z[blk + 32:blk + 64, :]
                            )
                            nc.sync.dma_start(
                                out=zs[blk + 32:blk + 64, :], in_=z[blk:blk + 32, :]
                            )
                        dv = dst[:, X, sc * QC:(sc + 1) * QC]
                        nc.vector.tensor_add(dv, tmp, zs)
                    return run

                def v_unit(jp):
                    def run():
                        pv = mmps.tile([P, QC], f32, tag="mm")
                        for j2 in range(2):
                            j = 2 * jp + j2
                            for kc in range(DIN_T):
                                nc.tensor.matmul(
                                    pv[:, j2 * 256:(j2 + 1) * 256],
                                    xTc[:, kc, j * P:(j + 1) * P],
                                    wv_sb[:, kc, :],
                                    start=(kc == 0 and j2 == 0),
                                    stop=(kc == DIN_T - 1 and j2 == 1),
                                )
                        for j2 in range(2):
                            st = sc * 4 + 2 * jp + j2
                            nc.scalar.copy(
                                out=v_sb[:, :, st, 0:64],
                                in_=pv[:, j2 * 256:(j2 + 1) * 256].rearrange(
                                    "p (h c) -> p h c", h=HPC
                                ),
                            )
                    return run

                units = [qk_unit(qT_sb, wq_sb, 0), qk_unit(qT_sb, wq_sb, 1),
                         qk_unit(kT_sb, wk_sb, 0), qk_unit(kT_sb, wk_sb, 1),
                         v_unit(0), v_unit(1)]
                return prefetch, units

            def attn_units(qc):
                q0 = qc * QC

                def head_unit(h):
                    g, o = h // 2, 64 * (h % 2)
                    qh = qT_sb[o:o + 64, g, :]
                    kh = kT_sb[o:o + 64, g, :]
                    O = pvps.tile([65, QC], f32, tag="O")
                    nfull = 4 * qc if is_causal else NKT
                    npair = nfull // 2
                    # software pipeline: scores(pr) ; PV(pr-1)
                    sc_t = [None] * npair
                    e_t = [None] * npair
                    for pr in range(npair):
                        sc2 = scps.tile([P, 2 * QC], f32, tag="sc")
                        sc_t[pr] = sc2
                        for half in range(2):
                            kt = 2 * pr + half
                            scv = sc2[:, half * QC:(half + 1) * QC]
                            nc.tensor.matmul(
                                scv,
                                kh[:, kt * P:(kt + 1) * P],
                                qh[:, q0:q0 + QC],
                                start=True,
                                stop=not use_kbias,
                            )
                            if use_kbias:
                                nc.tensor.matmul(
                                    scv,
                                    kbias_sb[:, kt * P:(kt + 1) * P],
                                    ones_q,
                                    start=False,
                                    stop=True,
                                )
                        e2 = epool.tile([P, 2 * QC], f16, tag="e2")
                        e_t[pr] = e2
                        nc.scalar.activation(out=e2, in_=sc2, func=EXP, scale=0.125)
                        if pr > 0:
                            for half in range(2):
                                kt = 2 * (pr - 1) + half
                                nc.tensor.matmul(
                                    O,
                                    v_sb[:, h, kt, 0:65],
                                    e_t[pr - 1][:, half * QC:(half + 1) * QC],
                                    start=(kt == 0),
                                    stop=False,
                                )
                    if is_causal:
                        # diag tiles dt=0..3, kt = 4qc+dt, valid q = [128dt, 512)
                        # slot A [P,1024]: dt0@0 (512) | dt1@512 (384) | dt3@896 (128)
                        # slot B [P,256]: dt2@0 (256); each + 128-wide triangle mask
                        kt0 = 4 * qc
                        scA = scps.tile([P, 2 * QC], f32, tag="sc")
                        offA = {0: 0, 1: 512, 3: 896}
                        nc.tensor.matmul(
                            scA[:, 0:512], kh[:, kt0 * P:(kt0 + 1) * P],
                            qh[:, q0:q0 + 512], start=True, stop=False,
                        )
                        nc.tensor.matmul(
                            scA[:, 0:128], ident_sb, mstd_sb, start=False, stop=True,
                        )
                        nc.tensor.matmul(
                            scA[:, 512:896], kh[:, (kt0 + 1) * P:(kt0 + 2) * P],
                            qh[:, q0 + 128:q0 + 512], start=True, stop=False,
                        )
                        nc.tensor.matmul(
                            scA[:, 512:640], ident_sb, mstd_sb, start=False, stop=False,
                        )
                        nc.tensor.matmul(
                            scA[:, 896:1024], kh[:, (kt0 + 3) * P:(kt0 + 4) * P],
                            qh[:, q0 + 384:q0 + 512], start=False, stop=False,
                        )
                        nc.tensor.matmul(
                            scA[:, 896:1024], ident_sb, mstd_sb, start=False, stop=True,
                        )
                        scB = scps.tile([P, 2 * QC], f32, tag="sc")
                        nc.tensor.matmul(
                            scB[:, 0:256], kh[:, (kt0 + 2) * P:(kt0 + 3) * P],
                            qh[:, q0 + 256:q0 + 512], start=True, stop=False,
                        )
                        nc.tensor.matmul(
                            scB[:, 0:128], ident_sb, mstd_sb, start=False, stop=True,
                        )
                        eA = epool.tile([P, 2 * QC], f16, tag="eA")
                        nc.scalar.activation(
                            out=eA, in_=scA, func=EXP, scale=0.125
                        )
                        eB = epool.tile([P, 2 * QC], f16, tag="eB")
                        nc.scalar.activation(
                            out=eB[:, 0:256], in_=scB[:, 0:256], func=EXP, scale=0.125
                        )
                        # drain the pipelined PV of the last full pair
                        if npair > 0:
                            for half in range(2):
                                kt = 2 * (npair - 1) + half
                                nc.tensor.matmul(
                                    O,
                                    v_sb[:, h, kt, 0:65],
                                    e_t[npair - 1][:, half * QC:(half + 1) * QC],
                                    start=(kt == 0),
                                    stop=False,
                                )
                        # diag PV: dt0 [0:512), dt1 [128:512), dt2 [256:512), dt3 [384:512)
                        nc.tensor.matmul(
                            O[:, 0:512], v_sb[:, h, kt0, 0:65], eA[:, 0:512],
                            start=(qc == 0), stop=False,
                        )
                        nc.tensor.matmul(
                            O[:, 128:512], v_sb[:, h, kt0 + 1, 0:65], eA[:, 512:896],
                            start=False, stop=False,
                        )
                        nc.tensor.matmul(
                            O[:, 256:512], v_sb[:, h, kt0 + 2, 0:65], eB[:, 0:256],
                            start=False, stop=False,
                        )
                        nc.tensor.matmul(
                            O[:, 384:512], v_sb[:, h, kt0 + 3, 0:65], eA[:, 896:1024],
                            start=False, stop=True,
                        )
                    else:
                        for half in range(2):
                            kt = 2 * (npair - 1) + half
                            nc.tensor.matmul(
                                O,
                                v_sb[:, h, kt, 0:65],
                                e_t[npair - 1][:, half * QC:(half + 1) * QC],
                                start=(kt == 0),
                                stop=(half == 1),
                            )
                    # normalize: r = 1/denom ; bc = broadcast(r) ; at = O * bc
                    r = ytile.tile([1, QC], f32r, tag="r")
                    nc.vector.reciprocal(r, O[64:65, :])
                    bc = mmps.tile([P, QC], f32, tag="mm")
                    nc.tensor.matmul(
                        bc[0:64, :], ones_bc, r, start=True, stop=True,
                    )
                    if o == 0:
                        at = attnT_sb[0:64, g, q0:q0 + QC]
                        nc.vector.tensor_copy(at, O[0:64, :])
                        nc.vector.tensor_mul(at, at, bc[0:64, :])
                    else:
                        st_odd = stage.tile([64, QC], f16, tag="stodd")
                        nc.vector.tensor_copy(st_odd, O[0:64, :])
                        nc.vector.tensor_mul(st_odd, st_odd, bc[0:64, :])
                        nc.sync.dma_start(
                            out=attnT_sb[64:128, g, q0:q0 + QC], in_=st_odd
                        )

                def outproj_unit(jpair):
                    def run():
                        for j in jpair:
                            st = qc * 4 + j
                            yt = ytile.tile([P, D], f16, tag="yt")
                            for nb in range(2):
                                yp = mmps.tile([P, QC], f32, tag="mm")
                                for g in range(2):
                                    nc.tensor.matmul(
                                        yp,
                                        attnT_sb[:, g, st * P:(st + 1) * P],
                                        wo_sb[:, g, nb * QC:(nb + 1) * QC],
                                        start=(g == 0),
                                        stop=(g == 1),
                                    )
                                nc.vector.tensor_copy(
                                    yt[:, nb * QC:(nb + 1) * QC], yp
                                )
                            nc.sync.dma_start(out=y[st * P:(st + 1) * P, :], in_=yt)
                    return run

                return [lambda h=h: head_unit(h) for h in (1, 3, 0, 2)] + [
                    outproj_unit((0, 1)), outproj_unit((2, 3))
                ]

            # interleave proj(it) units with attn(it-1) units so exp (ACT) work
            # spreads across the whole iteration instead of bunching after proj
            for it in range(NQC + 1):
                pre, punits = (None, []) if it >= NQC else proj_units(it)
                aunits = [] if it == 0 else attn_units(it - 1)
                if pre is not None:
                    pre()
                order = list(punits) + list(aunits)
                for u in order:
                    u()
            if debug:
                nc.sync.dma_start(out=dbg_qT[:, :, :], in_=qT_sb)
                nc.sync.dma_start(out=dbg_kT[:, :, :], in_=kT_sb)
                nc.sync.dma_start(out=dbg_v[:, :, :, :], in_=v_sb[:, :, :, 0:65])
                nc.sync.dma_start(out=dbg_at[:, :, :], in_=attnT_sb)

    _split_matmul_waits(nc, wfix_sem)
    return nc


def _split_matmul_waits(nc, wfix_sem):
    """Walrus's engine-instruction sync-wait slots are scarce (fp16 matmul
    takes exactly one; DVE/ACT structs also cap out). Leave one wait on the
    instruction and move the rest onto NoOps inserted just before it, each
    carrying a single wait."""
    import concourse.mybir as mybir
    import bass_rust

    n_fix = 0
    for blk in nc.m.functions[0].blocks:
        il = blk.instructions
        out = []
        changed = False
        for inst in il:
            si = inst.sync_info
            if si is not None and len(si.on_wait) > 1:
                merged = {}
                for w in si.on_wait:
                    k = (w.sync_type, w.id, w.wait_mode)
                    if (
                        k in merged
                        and w.wait_mode == "sem-ge-imm"
                        and w.wait_reg is None
                    ):
                        if w.wait_value > merged[k].wait_value:
                            merged[k] = w
                    elif k in merged:
                        merged[(k, len(merged))] = w
                    else:
                        merged[k] = w
                waits = list(merged.values())
                if len(waits) == 1:
                    si.on_wait = waits
                    out.append(inst)
                    continue
                for j, w in enumerate(waits[:-1]):
                    nop = mybir.InstNoOp(name=f"{inst.name}-wfix{j}")
                    nop.engine = inst.engine
                    upd = bass_rust.SyncUpdate(
                        sync_type="semaphore", id=wfix_sem.num,
                        ant_name=wfix_sem.name, update_mode="sem-inc",
                        update_value=1, update_reg=None,
                    )
                    nop.sync_info = bass_rust.SyncInfo(on_wait=[w], on_update=[upd])
                    out.append(nop)
                    n_fix += 1
                si.on_wait = [waits[-1]]
                changed = True
            out.append(inst)
        if changed:
            blk.instructions = out


def _host_tables():
    j = np.arange(32)
    inv_freq = (10000.0 ** (-j / 32.0)).astype(np.float64)
    ang = np.arange(S, dtype=np.float64)[:, None] * inv_freq[None, :]  # [S, 32]
    cosv = np.cos(ang).astype(np.float32).T   # [32, S]
    sinv = np.sin(ang).astype(np.float32).T
    C = np.empty((P, S), dtype=np.float32)
    T = np.empty((P, S), dtype=np.float32)
    for blk in (0, 64):
        C[blk:blk + 32] = cosv
        C[blk + 32:blk + 64] = cosv
        T[blk:blk + 32] = sinv          # lo rows carry +sin (headed to hi output)
        T[blk + 32:blk + 64] = -sinv    # hi rows carry -sin (headed to lo output)
    i = np.arange(P)[:, None]
    u = np.arange(P)[None, :]
    M = np.where(u >= i, 0.0, MBIG).astype(np.float16)  # strict lower = masked
    return C.astype(np.float16), T.astype(np.float16), M


def _in_maps(x, qkv_w, out_w, attn_mask, is_causal):
    C, T, M = _host_tables()
    ident = np.eye(P, dtype=np.float16)
    wq_full = qkv_w[:, 0:D]
    wk_full = qkv_w[:, D:2 * D]
    wv_full = qkv_w[:, 2 * D:3 * D]
    use_kbias = (not is_causal) and not bool(np.all(attn_mask))
    maps = []
    for core in range(8):
        b, hg = core // 4, core % 4
        cols = slice(hg * 256, (hg + 1) * 256)
        if use_kbias:
            kb = np.where(attn_mask[b], 0.0, MBIG).astype(np.float16)[None, :]
        else:
            kb = np.zeros((1, S), dtype=np.float16)
        maps.append(
            dict(
                xT=np.ascontiguousarray(x[b].T).astype(np.float16),
                wq=np.ascontiguousarray(wq_full[:, cols]).astype(np.float16),
                wk=np.ascontiguousarray(wk_full[:, cols]).astype(np.float16),
                wv=np.ascontiguousarray(wv_full[:, cols]).astype(np.float16),
                wo=np.ascontiguousarray(
                    out_w[hg * 256:(hg + 1) * 256, :].reshape(2, P, D)
                ).astype(np.float16),
                vones=np.ones((P, NKT), dtype=np.float16),
                onesb=np.ones((1, 64), dtype=np.float32),
                ctab=C,
                ttab=T,
                mstd=M,
                ident=ident,
                kbias=kb,
            )
        )
    return maps, use_kbias


def kernel(x, qkv_w, out_w, attn_mask, is_causal):
    from concourse.bass_utils import run_bass_kernel_spmd

    x = np.asarray(x, dtype=np.float32)
    qkv_w = np.asarray(qkv_w, dtype=np.float32)
    out_w = np.asarray(out_w, dtype=np.float32)
    attn_mask = np.asarray(attn_mask).astype(bool)
    causal = bool(np.asarray(is_causal))

    maps, use_kbias = _in_maps(x, qkv_w, out_w, attn_mask, causal)
    nc = _build_nc(causal, use_kbias)
    res = run_bass_kernel_spmd(nc, maps, list(range(8)))
    out = np.zeros((2, S, D), dtype=np.float32)
    for core in range(8):
        out[core // 4] += res.results[core]["y"].astype(np.float32)
    return out


# revision 50
# speedup vs baseline: 1.4761x; 1.0721x over previous
"""Fused causal attention block (qkv proj + RoPE + attention + out proj) on 8 TRN2 cores.

Sharding: data-parallel over batch (2) x tensor-parallel over heads (16 -> 4 per core).
Each core computes y_partial[b] = attn_heads_group(x[b]) @ out_w[group_rows]; the host
sums the 4 partials per batch (the out-projection "all-reduce") and stacks batches.

v2 layout (all matmul operands fp16, fp32 PSUM accumulation):
  - chunk-interleaved schedule: proj(chunk i) || attn(chunk i-1); causal q-chunk i only
    needs k-chunks <= i, so attention starts while later projections still run.
  - diagonal k-tiles computed at trimmed width (only q >= k-tile start) using PSUM
    pending-zero semantics; causal mask added as a 128-wide static lower-triangle
    table via identity matmul (N=128 instead of N=512 per diag tile).
  - out-projection packs head pairs: attnT [128=2x64 vdims, 2, S] against
    wo [128, 2, D] -> K=128 contraction, half the accumulation passes. Odd head's
    normalized attn rows are moved to partitions 64-127 by a small SBUF->SBUF DMA.
  - softmax denominator from a ones-column appended to v (row 64 of the PV psum);
    reciprocal on DVE, broadcast across 64 partitions by a K=1 matmul, applied in
    the same DVE multiply that writes attnT.
"""

import numpy as np

S = 2048
D = 1024
H = 16
DH = 64
P = 128
HPC = 4          # heads per core
QC = 512         # q-chunk width
NQC = S // QC
NKT = S // P     # k tiles
DIN_T = D // P   # contraction tiles for projections
MBIG = -60000.0  # pre-scale mask bias (fp16-safe); * 0.125 = -7500 -> exp == 0.0


DEBUG_NAMES = ["dbg_qT", "dbg_kT", "dbg_v", "dbg_at"]


def _build_nc(is_causal: bool, use_kbias: bool, debug: bool = False):
    import concourse.bass as bass
    import concourse.mybir as mybir
    import concourse.tile as tile

    f16 = mybir.dt.float16
    f32 = mybir.dt.float32
    f32r = mybir.dt.float32r
    EXP = mybir.ActivationFunctionType.Exp

    nc = bass.Bass()
    wfix_sem = nc.alloc_semaphore("wfix")
    xT = nc.dram_tensor("xT", [D, S], f16, kind="ExternalInput")
    wq = nc.dram_tensor("wq", [D, 256], f16, kind="ExternalInput")
    wk = nc.dram_tensor("wk", [D, 256], f16, kind="ExternalInput")
    wv = nc.dram_tensor("wv", [D, 256], f16, kind="ExternalInput")
    wo = nc.dram_tensor("wo", [2, P, D], f16, kind="ExternalInput")
    ctab = nc.dram_tensor("ctab", [P, S], f16, kind="ExternalInput")
    ttab = nc.dram_tensor("ttab", [P, S], f16, kind="ExternalInput")
    kbias = nc.dram_tensor("kbias", [1, S], f16, kind="ExternalInput")
    vones = nc.dram_tensor("vones", [P, NKT], f16, kind="ExternalInput")
    onesb = nc.dram_tensor("onesb", [1, 64], f32r, kind="ExternalInput")
    onesq_in = nc.dram_tensor("onesq_in", [1, QC], f16, kind="ExternalInput")
    tmask = nc.dram_tensor("tmask", [P, P], f16, kind="ExternalInput")
    y = nc.dram_tensor("y", [S, D], f16, kind="ExternalOutput")
    if debug:
        dbg_qT = nc.dram_tensor("dbg_qT", [P, 2, S], f16, kind="ExternalOutput")
        dbg_kT = nc.dram_tensor("dbg_kT", [P, 2, S], f16, kind="ExternalOutput")
        dbg_v = nc.dram_tensor("dbg_v", [P, HPC, NKT, 65], f16, kind="ExternalOutput")
        dbg_at = nc.dram_tensor("dbg_at", [P, 2, S], f16, kind="ExternalOutput")

    with tile.TileContext(nc) as tc, nc.allow_low_precision(
        reason="fp16 operands with fp32 PSUM accumulation; rel-err budget 2e-2"
    ):
        with (
            tc.tile_pool(name="pers", bufs=1) as pers,
            tc.tile_pool(name="xpool", bufs=1) as xpool,
            tc.tile_pool(name="ropet", bufs=4) as ropet,
            tc.tile_pool(name="epool", bufs=6) as epool,
            tc.tile_pool(name="stage", bufs=4) as stage,
            tc.tile_pool(name="ytile", bufs=6) as ytile,
            tc.tile_pool(name="scps", bufs=2, space="PSUM") as scps,
            tc.tile_pool(name="pvps", bufs=2, space="PSUM") as pvps,
            tc.tile_pool(name="mmps", bufs=2, space="PSUM") as mmps,
        ):
            qT_sb = pers.tile([P, 2, S], f16, tag="qT")
            kT_sb = pers.tile([P, 2, S], f16, tag="kT")
            v_sb = pers.tile([P, HPC, NKT, 66], f16, tag="v")
            attnT_sb = pers.tile([P, 2, S], f16, tag="attnT")
            wo_sb = pers.tile([P, 2, D], f16, tag="wo")
            tmask_sb = pers.tile([P, P], f16, tag="tmask")
            ones_bc = pers.tile([1, 64], f32r, tag="onesbc")
            wq_sb = pers.tile([P, DIN_T, 256], f16, tag="wq")
            wk_sb = pers.tile([P, DIN_T, 256], f16, tag="wk")
            wv_sb = pers.tile([P, DIN_T, 256], f16, tag="wv")
            c_sb = pers.tile([P, S], f16, tag="ctab")
            t_sb = pers.tile([P, S], f16, tag="ttab")
            x_ch = [
                xpool.tile([P, DIN_T, QC], f16, name=f"x{sc}", tag=f"x{sc}")
                for sc in range(NQC)
            ]
            if use_kbias:
                kbias_sb = pers.tile([1, S], f16, tag="kbias")
                ones_q = pers.tile([1, QC], f16, tag="onesq")
                nc.sync.dma_start(out=kbias_sb, in_=kbias[:, :])
                nc.sync.dma_start(
                    out=ones_q, in_=onesq_in[:, :]
                )
            # ---- prologue DMAs (one per tensor: HWDGE issue costs 625ns/DMA) ----
            # q-projection critical path first: x0, wq, then wk, rope tables,
            # wv, the chunk-1 prefetch, and the rest.
            def x_load(sc):
                nc.sync.dma_start(
                    out=x_ch[sc],
                    in_=xT[:, sc * QC:(sc + 1) * QC].rearrange(
                        "(t p) f -> p t f", p=P
                    ),
                )

            for lo, hi in ((0, 1), (1, 2), (2, 4), (4, 8)):
                nc.sync.dma_start(
                    out=x_ch[0][:, lo:hi, :],
                    in_=xT[lo * P:hi * P, 0:QC].rearrange(
                        "(t p) f -> p t f", p=P
                    ),
                )
                nc.sync.dma_start(
                    out=wq_sb[:, lo:hi, :],
                    in_=wq[lo * P:hi * P, :].rearrange(
                        "(t p) n -> p t n", p=P
                    ),
                )
            nc.sync.dma_start(out=wk_sb, in_=wk.rearrange("(t p) n -> p t n", p=P))
            nc.sync.dma_start(out=c_sb, in_=ctab[:, :])
            nc.sync.dma_start(out=t_sb, in_=ttab[:, :])
            nc.sync.dma_start(out=wv_sb, in_=wv.rearrange("(t p) n -> p t n", p=P))
            x_load(1)
            nc.sync.dma_start(out=tmask_sb, in_=tmask[:, :])
            nc.sync.dma_start(out=ones_bc, in_=onesb[:, :])
            for g in range(2):
                nc.sync.dma_start(out=wo_sb[:, g, :], in_=wo[g, :, :])
            for h in range(HPC):
                nc.sync.dma_start(
                    out=v_sb[:, h, :, 64:65],
                    in_=vones.rearrange("p (k o) -> p k o", o=1),
                )

            def proj_units(sc):
                xTc = x_ch[sc]
                cs = c_sb[:, sc * QC:(sc + 1) * QC]
                ts = t_sb[:, sc * QC:(sc + 1) * QC]

                def prefetch():
                    if sc + 2 < NQC:
                        x_load(sc + 2)

                rope_adds = []

                def qk_unit(dst, w_sb, X):
                    def run():
                        pq = mmps.tile([P, QC], f32, tag="mm")
                        for kc in range(DIN_T):
                            nc.tensor.matmul(
                                pq,
                                w_sb[:, kc, X * P:(X + 1) * P],
                                xTc[:, kc, :],
                                start=(kc == 0),
                                stop=(kc == DIN_T - 1),
                            )
                        tmp = ropet.tile([P, QC], f16, tag="tmp")
                        z = ropet.tile([P, QC], f16, tag="z")
                        zs = ropet.tile([P, QC], f16, tag="zs")
                        nc.vector.tensor_mul(tmp, pq, cs)
                        nc.vector.tensor_mul(z, pq, ts)
                        for blk in (0, 64):
                            nc.sync.dma_start(
                                out=zs[blk:blk + 32, :], in_=z[blk + 32:blk + 64, :]
                            )
                            nc.sync.dma_start(
                                out=zs[blk + 32:blk + 64, :], in_=z[blk:blk + 32, :]
                            )
                        dv = dst[:, X, sc * QC:(sc + 1) * QC]

                        def add():
                            nc.vector.tensor_add(dv, tmp, zs)
                        rope_adds.append(add)
                    return run

                def v_unit(jp):
                    def run():
                        pv = mmps.tile([P, QC], f32, tag="mm")
                        for j2 in range(2):
                            j = 2 * jp + j2
                            for kc in range(DIN_T):
                                nc.tensor.matmul(
                                    pv[:, j2 * 256:(j2 + 1) * 256],
                                    xTc[:, kc, j * P:(j + 1) * P],
                                    wv_sb[:, kc, :],
                                    start=(kc == 0 and j2 == 0),
                                    stop=(kc == DIN_T - 1 and j2 == 1),
                                )
                        for j2 in range(2):
                            st = sc * 4 + 2 * jp + j2
                            nc.scalar.copy(
                                out=v_sb[:, :, st, 0:64],
                                in_=pv[:, j2 * 256:(j2 + 1) * 256].rearrange(
                                    "p (h c) -> p h c", h=HPC
                                ),
                            )
                    return run

                units = [qk_unit(qT_sb, wq_sb, 0), qk_unit(qT_sb, wq_sb, 1),
                         qk_unit(kT_sb, wk_sb, 0), qk_unit(kT_sb, wk_sb, 1)]
                vunits = [v_unit(0), v_unit(1)]
                return prefetch, units, vunits, rope_adds

            def attn_units(qc):
                q0 = qc * QC

                def head_unit(h):
                    g, o = h // 2, 64 * (h % 2)
                    qh = qT_sb[o:o + 64, g, :]
                    kh = kT_sb[o:o + 64, g, :]
                    O = pvps.tile([65, QC], f32, tag="O")
                    nfull = 4 * qc if is_causal else NKT
                    npair = nfull // 2
                    if is_causal:
                        # diag tiles first: their exps enter the ACT queue early
                        # so the diag PVs at the head's end never wait.
                        # slot A [P,1024]: dt0@0 (512) | dt1@512 (384) | dt3@896 (128)
                        # slot B [P,256]: dt2@0 (256); each + 128-wide triangle mask
                        kt0 = 4 * qc
                        scA = scps.tile([P, 2 * QC], f32, tag="sc")
                        nc.tensor.matmul(
                            scA[:, 0:512], kh[:, kt0 * P:(kt0 + 1) * P],
                            qh[:, q0:q0 + 512], start=True, stop=True,
                        )
                        nc.tensor.matmul(
                            scA[:, 512:896], kh[:, (kt0 + 1) * P:(kt0 + 2) * P],
                            qh[:, q0 + 128:q0 + 512], start=True, stop=False,
                        )
                        nc.tensor.matmul(
                            scA[:, 896:1024], kh[:, (kt0 + 3) * P:(kt0 + 4) * P],
                            qh[:, q0 + 384:q0 + 512], start=False, stop=True,
                        )
                        scB = scps.tile([P, 2 * QC], f32, tag="sc")
                        nc.tensor.matmul(
                            scB[:, 0:256], kh[:, (kt0 + 2) * P:(kt0 + 3) * P],
                            qh[:, q0 + 256:q0 + 512], start=True, stop=True,
                        )
                        eA = epool.tile([P, 2 * QC], f16, tag="eA")
                        nc.scalar.activation(
                            out=eA, in_=scA, func=EXP, scale=0.125
                        )
                        eB = epool.tile([P, 2 * QC], f16, tag="eB")
                        nc.scalar.activation(
                            out=eB[:, 0:256], in_=scB[:, 0:256], func=EXP, scale=0.125
                        )
                        # multiplicative causal mask on the diag-block columns
                        nc.vector.tensor_mul(
                            eA[:, 0:128], eA[:, 0:128], tmask_sb
                        )
                        nc.vector.tensor_mul(
                            eA[:, 512:640], eA[:, 512:640], tmask_sb
                        )
                        nc.vector.tensor_mul(
                            eA[:, 896:1024], eA[:, 896:1024], tmask_sb
                        )
                        nc.vector.tensor_mul(
                            eB[:, 0:128], eB[:, 0:128], tmask_sb
                        )
                    # software pipeline: scores(pr) ; PV(pr-1)
                    sc_t = [None] * npair
                    e_t = [None] * npair
                    for pr in range(npair):
                        sc2 = scps.tile([P, 2 * QC], f32, tag="sc")
                        sc_t[pr] = sc2
                        for half in range(2):
                            kt = 2 * pr + half
                            scv = sc2[:, half * QC:(half + 1) * QC]
                            nc.tensor.matmul(
                                scv,
                                kh[:, kt * P:(kt + 1) * P],
                                qh[:, q0:q0 + QC],
                                start=True,
                                stop=not use_kbias,
                            )
                            if use_kbias:
                                nc.tensor.matmul(
                                    scv,
                                    kbias_sb[:, kt * P:(kt + 1) * P],
                                    ones_q,
                                    start=False,
                                    stop=True,
                                )
                        e2 = epool.tile([P, 2 * QC], f16, tag="e2")
                        e_t[pr] = e2
                        nc.scalar.activation(out=e2, in_=sc2, func=EXP, scale=0.125)
                        if pr > 0:
                            for half in range(2):
                                kt = 2 * (pr - 1) + half
                                nc.tensor.matmul(
                                    O,
                                    v_sb[:, h, kt, 0:65],
                                    e_t[pr - 1][:, half * QC:(half + 1) * QC],
                                    start=(kt == 0),
                                    stop=False,
                                )
                    if is_causal:
                        # drain the pipelined PV of the last full pair
                        if npair > 0:
                            for half in range(2):
                                kt = 2 * (npair - 1) + half
                                nc.tensor.matmul(
                                    O,
                                    v_sb[:, h, kt, 0:65],
                                    e_t[npair - 1][:, half * QC:(half + 1) * QC],
                                    start=(kt == 0),
                                    stop=False,
                                )
                        # diag PV: dt0 [0:512), dt1 [128:512), dt2 [256:512), dt3 [384:512)
                        nc.tensor.matmul(
                            O[:, 0:512], v_sb[:, h, kt0, 0:65], eA[:, 0:512],
                            start=(qc == 0), stop=False,
                        )
                        nc.tensor.matmul(
                            O[:, 128:512], v_sb[:, h, kt0 + 1, 0:65], eA[:, 512:896],
                            start=False, stop=False,
                        )
                        nc.tensor.matmul(
                            O[:, 256:512], v_sb[:, h, kt0 + 2, 0:65], eB[:, 0:256],
                            start=False, stop=False,
                        )
                        nc.tensor.matmul(
                            O[:, 384:512], v_sb[:, h, kt0 + 3, 0:65], eA[:, 896:1024],
                            start=False, stop=True,
                        )
                    else:
                        for half in range(2):
                            kt = 2 * (npair - 1) + half
                            nc.tensor.matmul(
                                O,
                                v_sb[:, h, kt, 0:65],
                                e_t[npair - 1][:, half * QC:(half + 1) * QC],
                                start=(kt == 0),
                                stop=(half == 1),
                            )
                    # normalize: r = 1/denom ; bc = broadcast(r) ; at = O * bc
                    r = ytile.tile([1, QC], f32r, tag="r")
                    nc.vector.reciprocal(r, O[64:65, :])
                    bc = mmps.tile([P, QC], f32, tag="mm")
                    nc.tensor.matmul(
                        bc[0:64, :], ones_bc, r, start=True, stop=True,
                    )
                    if o == 0:
                        at = attnT_sb[0:64, g, q0:q0 + QC]
                        nc.vector.tensor_copy(at, O[0:64, :])
                        nc.vector.tensor_mul(at, at, bc[0:64, :])
                    else:
                        st_odd = stage.tile([64, QC], f16, tag="stodd")
                        nc.vector.tensor_copy(st_odd, O[0:64, :])
                        nc.vector.tensor_mul(st_odd, st_odd, bc[0:64, :])
                        nc.sync.dma_start(
                            out=attnT_sb[64:128, g, q0:q0 + QC], in_=st_odd
                        )

                def outproj_unit(jpair):
                    def run():
                        for j in jpair:
                            st = qc * 4 + j
                            yt = ytile.tile([P, D], f16, tag="yt")
                            for nb in range(2):
                                yp = mmps.tile([P, QC], f32, tag="mm")
                                for g in range(2):
                                    nc.tensor.matmul(
                                        yp,
                                        attnT_sb[:, g, st * P:(st + 1) * P],
                                        wo_sb[:, g, nb * QC:(nb + 1) * QC],
                                        start=(g == 0),
                                        stop=(g == 1),
                                    )
                                nc.vector.tensor_copy(
                                    yt[:, nb * QC:(nb + 1) * QC], yp
                                )
                                nc.sync.dma_start(
                                    out=y[st * P:(st + 1) * P,
                                          nb * QC:(nb + 1) * QC],
                                    in_=yt[:, nb * QC:(nb + 1) * QC],
                                )
                    return run

                return ([lambda h=h: head_unit(h) for h in (1, 3, 0, 2)],
                        [outproj_unit((0, 1)), outproj_unit((2, 3))])

            # interleave proj(it) units with attn(it-1) units so exp (ACT) work
            # spreads across the whole iteration instead of bunching after proj
            if not is_causal:
                # non-causal attention needs every k chunk: run all
                # projections first, then all attention chunks.
                for sc in range(NQC):
                    pre, punits, vunits, radds = proj_units(sc)
                    pre()
                    for u in punits + vunits:
                        u()
                    for a in radds:
                        a()
                for qc in range(NQC):
                    aunits, opunits = attn_units(qc)
                    for u in aunits + opunits:
                        u()
                punits = []
            pend = []
            for it in range(NQC + 1 if is_causal else 0):
                pre, punits, vunits, radds = (
                    (None, [], [], []) if it >= NQC else proj_units(it)
                )
                if it == 0:
                    aunits, opunits = [], []
                else:
                    aunits, opunits = attn_units(it - 1)
                opunits, pend = pend, opunits
                for u in punits[:2]:
                    u()
                for u in aunits[:1]:
                    u()
                for u in punits[2:]:
                    u()
                for u in opunits[:1]:
                    u()
                for u in aunits[1:2]:
                    u()
                for u in vunits[:1]:
                    u()
                for u in opunits[1:]:
                    u()
                for u in aunits[2:3]:
                    u()
                for u in vunits[1:]:
                    u()
                for u in aunits[3:]:
                    u()
                if pre is not None:
                    pre()
                for a in radds:
                    a()
            for u in pend:
                u()
            if debug:
                nc.sync.dma_start(out=dbg_qT[:, :, :], in_=qT_sb)
                nc.sync.dma_start(out=dbg_kT[:, :, :], in_=kT_sb)
                nc.sync.dma_start(out=dbg_v[:, :, :, :], in_=v_sb[:, :, :, 0:65])
                nc.sync.dma_start(out=dbg_at[:, :, :], in_=attnT_sb)

    _split_matmul_waits(nc, wfix_sem)
    return nc


def _split_matmul_waits(nc, wfix_sem):
    """Walrus's engine-instruction sync-wait slots are scarce (fp16 matmul
    takes exactly one; DVE/ACT structs also cap out). Leave one wait on the
    instruction and move the rest onto NoOps inserted just before it, each
    carrying a single wait."""
    import concourse.mybir as mybir
    import bass_rust

    n_fix = 0
    for blk in nc.m.functions[0].blocks:
        il = blk.instructions
        out = []
        changed = False
        for inst in il:
            si = inst.sync_info
            if si is not None and len(si.on_wait) > 1:
                merged = {}
                for w in si.on_wait:
                    k = (w.sync_type, w.id, w.wait_mode)
                    if (
                        k in merged
                        and w.wait_mode == "sem-ge-imm"
                        and w.wait_reg is None
                    ):
                        if w.wait_value > merged[k].wait_value:
                            merged[k] = w
                    elif k in merged:
                        merged[(k, len(merged))] = w
                    else:
                        merged[k] = w
                waits = list(merged.values())
                if len(waits) == 1:
                    si.on_wait = waits
                    out.append(inst)
                    continue
                for j, w in enumerate(waits[:-1]):
                    nop = mybir.InstNoOp(name=f"{inst.name}-wfix{j}")
                    nop.engine = inst.engine
                    upd = bass_rust.SyncUpdate(
                        sync_type="semaphore", id=wfix_sem.num,
                        ant_name=wfix_sem.name, update_mode="sem-inc",
                        update_value=1, update_reg=None,
                    )
                    nop.sync_info = bass_rust.SyncInfo(on_wait=[w], on_update=[upd])
                    out.append(nop)
                    n_fix += 1
                si.on_wait = [waits[-1]]
                changed = True
            out.append(inst)
        if changed:
            blk.instructions = out


def _host_tables():
    j = np.arange(32)
    inv_freq = (10000.0 ** (-j / 32.0)).astype(np.float64)
    ang = np.arange(S, dtype=np.float64)[:, None] * inv_freq[None, :]  # [S, 32]
    cosv = np.cos(ang).astype(np.float32).T   # [32, S]
    sinv = np.sin(ang).astype(np.float32).T
    C = np.empty((P, S), dtype=np.float32)
    T = np.empty((P, S), dtype=np.float32)
    for blk in (0, 64):
        C[blk:blk + 32] = cosv
        C[blk + 32:blk + 64] = cosv
        T[blk:blk + 32] = sinv          # lo rows carry +sin (headed to hi output)
        T[blk + 32:blk + 64] = -sinv    # hi rows carry -sin (headed to lo output)
    i = np.arange(P)[:, None]
    u = np.arange(P)[None, :]
    M = np.where(u >= i, 1.0, 0.0).astype(np.float16)  # 0/1 multiplicative mask
    return C.astype(np.float16), T.astype(np.float16), M


def _in_maps(x, qkv_w, out_w, attn_mask, is_causal):
    C, T, M = _host_tables()
    wq_full = qkv_w[:, 0:D]
    wk_full = qkv_w[:, D:2 * D]
    wv_full = qkv_w[:, 2 * D:3 * D]
    use_kbias = (not is_causal) and not bool(np.all(attn_mask))
    maps = []
    for core in range(8):
        b, hg = core // 4, core % 4
        cols = slice(hg * 256, (hg + 1) * 256)
        if use_kbias:
            kb = np.where(attn_mask[b], 0.0, MBIG).astype(np.float16)[None, :]
        else:
            kb = np.zeros((1, S), dtype=np.float16)
        maps.append(
            dict(
                xT=np.ascontiguousarray(x[b].T).astype(np.float16),
                wq=np.ascontiguousarray(wq_full[:, cols]).astype(np.float16),
                wk=np.ascontiguousarray(wk_full[:, cols]).astype(np.float16),
                wv=np.ascontiguousarray(wv_full[:, cols]).astype(np.float16),
                wo=np.ascontiguousarray(
                    out_w[hg * 256:(hg + 1) * 256, :].reshape(2, P, D)
                ).astype(np.float16),
                vones=np.ones((P, NKT), dtype=np.float16),
                onesb=np.ones((1, 64), dtype=np.float32),
                onesq_in=np.ones((1, QC), dtype=np.float16),
                ctab=C,
                ttab=T,
                tmask=M,
                kbias=kb,
            )
        )
    return maps, use_kbias


def kernel(x, qkv_w, out_w, attn_mask, is_causal):
    from concourse.bass_utils import run_bass_kernel_spmd

    x = np.asarray(x, dtype=np.float32)
    qkv_w = np.asarray(qkv_w, dtype=np.float32)
    out_w = np.asarray(out_w, dtype=np.float32)
    attn_mask = np.asarray(attn_mask).astype(bool)
    causal = bool(np.asarray(is_causal))

    maps, use_kbias = _in_maps(x, qkv_w, out_w, attn_mask, causal)
    nc = _build_nc(causal, use_kbias)
    res = run_bass_kernel_spmd(nc, maps, list(range(8)))
    out = np.zeros((2, S, D), dtype=np.float32)
    for core in range(8):
        out[core // 4] += res.results[core]["y"].astype(np.float32)
    return out
